# revision 37
# baseline (speedup 1.0000x reference)
"""Trainium2 Bass kernel for a 4-layer GraphConv GNN (DGL norm='both') + max-pool + FC + softmax.

Strategy (8 NeuronCores, SPMD, node/edge partitioned):
  - Nodes dealt round-robin to cores (core = n % 8); per-core nodes laid out on a
    [128 partitions x 98 chunks] grid (local l = j*128 + p). Edges sharded by dst.
  - Per layer, each core computes z = (h * ninv_out) @ W for its nodes (PE),
    publishes the z-shard (bf16) and AllGathers the full 100352-row z-table.
  - Message gather: InstDMAGatherAnt (SWDGE Q7 bulk gather, 4 rotating queues,
    <=1024 int16 indices per instruction). The int16 index range is solved by
    gathering 256B elements = 4 consecutive bf16 table rows with a per-phase
    (srow % 4) byte-offset window, so idx = srow >> 2 < 25089. Only the first
    32 bf16 of each gathered element are consumed.
  - Segment-sum: per dst-chunk PE matmul accumulation m_chunk += S_c^T @ slab_c
    over the chunk's slab columns, with host-precomputed 0/1 segment matrices
    S (fp8, streamed from HBM). No per-dst uniform-width padding needed; edge
    slots are padded only to 128-multiples per (chunk, phase) group (~12%).
  - Epilogue per chunk: h = relu(m * ninv_in + b); degrees are computed
    on-device by reducing 0/1 edge-existence masks (as unary encodings).
  - Final graph max-pool: PE transpose + reduce_max, AllReduce(max), tiny FC +
    softmax replicated on every core.

Perf notes (measured on HW via burst timing):
  - dma_gather marginal cost: 8.25 ns/idx on one SWDGE queue; 2.0 ns/idx with
    num_swdge_queues=4 and round-robin queue assignment (the per-queue
    trigger/ring path is the serial resource, and 4 queues scale it).
  - num_idxs > 1024 per dma_gather instruction wedges the device with
    single_packet=True (NRT unrecoverable); single_packet=False lifts the cap
    (2048/8192 verified) but 8192-idx instructions measure ~20% slower
    (coarser queue parallelism/overlap), 2048 is noise-equivalent. Keep
    1024-idx single-packet instructions.
  - dma_scatter_add loses updates for duplicate indices within one instruction
    (verified on HW), so the scatter-based segment-sum was abandoned.
  - The old per-edge indirect-DMA path costs 994ns SWDGE fixed overhead per
    128-descriptor instruction (~7.8 ns/edge engine-serial) - that was the
    18.2ms baseline's wall.
  - Issuing the S-matrix loads on the activation engine's HWDGE queue
    (nc.scalar.dma_start) instead of the sync engine's cut total time from
    ~3.0ms to ~2.4ms: the SP queue was serializing S loads behind idx loads
    and the z publish.
"""

import os
import sys

sys.path.insert(0, "/opt/trn_rl_repo")

import numpy as np
import ml_dtypes

import concourse.bass as bass
import concourse.bacc as bacc
import concourse.tile as tile
from concourse import mybir
from concourse.masks import make_identity
from concourse import bass_utils

F32 = mybir.dt.float32
BF16 = mybir.dt.bfloat16
FP8 = mybir.dt.float8e4
I16 = mybir.dt.int16

N_NODES = 100000
N_EDGES = 3200000
IN_DIM = 128
HID = 32
N_CLASSES = 8
N_CORES = 8
P = 128
J = 98                      # chunks of 128 nodes per core
SLOT = P * J                # 12544 table rows per core
REAL = N_NODES // N_CORES   # 12500 real nodes per core
NT = N_CORES * SLOT + 8     # z-table rows (+ pad for 4-row element overrun)
MAX_NI = 1024               # HW limit: dma_gather indices per instruction
NQ = 4                      # SWDGE queues

NP_FP8 = mybir.dt.np(FP8)
NP_BF16 = ml_dtypes.bfloat16

LAST_RESULTS = None
LAST_NC = None
LAST_IN_MAPS = None


# --------------------------------------------------------------------------
# Host-side layout planning (graph-structure preprocessing only)
# --------------------------------------------------------------------------
def wrap16_into(dstarr, col0, vals):
    """Write vals (len multiple of 16) into dstarr[:, col0:...] in the SWDGE
    idx layout: idx i -> [16k + i%16, col0 + i//16] for k in 0..7."""
    n = len(vals)
    cols = n // 16
    blk = np.asarray(vals, np.int16).reshape(cols, 16).T  # [16, cols]
    for k in range(8):
        dstarr[16 * k:16 * (k + 1), col0:col0 + cols] = blk


def make_plan(src, dst):
    src = np.asarray(src).astype(np.int64)
    dst = np.asarray(dst).astype(np.int64)

    core = dst % N_CORES
    # node n -> (core, p, j); local l = j*128 + p; table row = core*SLOT + p*J + j
    def node_pj(n):
        l = n // N_CORES
        return l % P, l // P

    p_s, j_s = node_pj(src)
    srow = (src % N_CORES) * SLOT + p_s * J + j_s
    p_d, j_d = node_pj(dst)

    deg_in = np.bincount(dst, minlength=N_NODES)
    deg_out = np.bincount(src, minlength=N_NODES)

    # ---- per-(core, chunk, phase) edge grouping ----
    q_e = srow % 4
    # group key: core * (J*4) + j_d * 4 + q
    gkey = core * (J * 4) + j_d * 4 + q_e
    order = np.lexsort((srow, gkey))
    gkey_s = gkey[order]
    srow_s = srow[order]
    pd_s = p_d[order]

    counts = np.bincount(gkey_s, minlength=N_CORES * J * 4).reshape(
        N_CORES, J, 4)
    nmax = counts.max(axis=0)                      # [J, 4]
    n_jq = ((nmax + 15) // 16) * 16                # group sizes (16-aligned)

    # Slab columns are RANGE-BLOCKED: ranges of G chunks; within a range the
    # 4 phases are contiguous runs, so gather instructions are long 1024-idx
    # streams that only break at (range, phase) boundaries. Groups within a
    # run are packed back-to-back at 16-idx granularity; chunks share
    # boundary slab columns (their S matrices are zero for foreign edges).
    # Only run ends are padded to 128. S stays CHUNK-MAJOR.
    G = 6
    ranges = [(r0, min(r0 + G, J)) for r0 in range(0, J, G)]
    o_jq = np.zeros((J, 4), dtype=np.int64)        # edge-slot offset of (j, q)
    cc = 0                                          # edge-slot counter
    range_meta = []                                 # (j0, j1, col0, [(q, e0, e1)])
    for (j0, j1) in ranges:
        rc0 = cc // 128
        runs = []
        for q in range(4):
            e0 = cc
            for j in range(j0, j1):
                o_jq[j, q] = cc
                cc += int(n_jq[j, q])
            cc = ((cc + 127) // 128) * 128          # pad run end to a column
            runs.append((q, e0, cc))
        range_meta.append((j0, j1, rc0, runs))
    TOTC = cc // 128

    # chunk-major S column lists: per (j, q) the [cstart, cend] column range
    chunk_cols = []                                 # [J] -> list of (q, cstart, cend)
    W = np.zeros(J, dtype=np.int64)
    for j in range(J):
        lst = []
        for q in range(4):
            n = int(n_jq[j, q])
            if n == 0:
                continue
            cstart = int(o_jq[j, q]) // 128
            cend = (int(o_jq[j, q]) + n - 1) // 128
            lst.append((q, cstart, cend))
            W[j] += cend - cstart + 1
        chunk_cols.append(lst)
    scol0 = np.zeros(J + 1, dtype=np.int64)        # chunk-major S offsets
    scol0[1:] = np.cumsum(W)

    GC = TOTC * 8  # gidx columns (128 idxs -> 8 cols of 16)
    SCOLS = int(scol0[J])  # chunk-major S columns (boundary cols duplicated)

    # ---- per-core gidx + segment matrices ----
    gidx = np.zeros((N_CORES, 128, GC), dtype=np.int16)
    sseg = np.zeros((N_CORES, 128, SCOLS * 128), dtype=NP_FP8)
    one = NP_FP8(1.0)

    gstart = np.zeros(N_CORES * J * 4 + 1, dtype=np.int64)
    gstart[1:] = np.cumsum(counts.reshape(-1))
    for c in range(N_CORES):
        for j in range(J):
            koff = 0  # chunk-local S column offset (q-major)
            for q in range(4):
                k = c * (J * 4) + j * 4 + q
                s0, s1 = int(gstart[k]), int(gstart[k + 1])
                n_real = s1 - s0
                n_pad = int(n_jq[j, q])
                if n_pad == 0:
                    continue
                o0 = int(o_jq[j, q])
                gi = np.zeros(n_pad, dtype=np.int16)
                gi[:n_real] = (srow_s[s0:s1] >> 2).astype(np.int16)
                wrap16_into(gidx[c], o0 // 16, gi)
                # S entries: stream position g = o0 + i -> slab column g//128,
                # partition g%128; chunk-local S column = koff + (g//128 -
                # cstart), matching the matmul traversal below.
                cstart = o0 // 128
                g = o0 + np.arange(n_real)
                kloc = koff + g // 128 - cstart
                epart = g % 128
                sseg[c, epart, (int(scol0[j]) + kloc) * 128 + pd_s[s0:s1]] = one
                koff += (o0 + n_pad - 1) // 128 - cstart + 1

    # ---- degree masks (unary encodings, reduced on device) ----
    # node (p, j) of core c is n = 8*(j*128+p) + c
    pp, jj = np.meshgrid(np.arange(P), np.arange(J), indexing="ij")
    l_grid = jj * 128 + pp                          # [P, J]
    valid = l_grid < REAL
    DP = np.zeros(J, dtype=np.int64)
    DPo = np.zeros(J, dtype=np.int64)
    deg_in_g = np.zeros((N_CORES, P, J), dtype=np.int64)
    deg_out_g = np.zeros((N_CORES, P, J), dtype=np.int64)
    for c in range(N_CORES):
        n_grid = np.where(valid, l_grid * N_CORES + c, 0)
        deg_in_g[c] = np.where(valid, deg_in[n_grid], 0)
        deg_out_g[c] = np.where(valid, deg_out[n_grid], 0)
    DP = deg_in_g.max(axis=(0, 1))                  # [J]
    DPo = deg_out_g.max(axis=(0, 1))
    DP = np.maximum(DP, 1)
    DPo = np.maximum(DPo, 1)
    off = np.zeros(J + 1, dtype=np.int64)
    off[1:] = np.cumsum(DP)
    offo = np.zeros(J + 1, dtype=np.int64)
    offo[1:] = np.cumsum(DPo)
    SD, SDo = int(off[J]), int(offo[J])

    mask_in = np.zeros((N_CORES, P, SD), dtype=NP_BF16)
    mask_out = np.zeros((N_CORES, P, SDo), dtype=NP_BF16)
    for c in range(N_CORES):
        for j in range(J):
            w = int(DP[j])
            ar = np.arange(w)[None, :]
            mask_in[c, :, int(off[j]):int(off[j]) + w] = (
                ar < deg_in_g[c, :, j:j + 1]).astype(NP_BF16)
            w = int(DPo[j])
            ar = np.arange(w)[None, :]
            mask_out[c, :, int(offo[j]):int(offo[j]) + w] = (
                ar < deg_out_g[c, :, j:j + 1]).astype(NP_BF16)

    return dict(
        range_meta=range_meta, W=W, chunk_cols=chunk_cols, scol0=scol0,
        TOTC=TOTC, GC=GC, SCOLS=SCOLS, gidx=gidx, sseg=sseg,
        DP=DP, DPo=DPo, off=off, offo=offo, SD=SD, SDo=SDo,
        mask_in=mask_in, mask_out=mask_out,
    )


# --------------------------------------------------------------------------
# Bass program
# --------------------------------------------------------------------------
def build_program(plan):
    range_meta = plan["range_meta"]
    W = plan["W"]
    chunk_cols = plan["chunk_cols"]
    scol0 = plan["scol0"]
    TOTC, GC = plan["TOTC"], plan["GC"]
    DP, DPo = plan["DP"], plan["DPo"]
    off, offo = plan["off"], plan["offo"]
    SD, SDo = plan["SD"], plan["SDo"]
    H = HID

    nc = bacc.Bacc("TRN2", target_bir_lowering=False, debug=False,
                   num_devices=N_CORES, num_swdge_queues=NQ)

    x_d = nc.dram_tensor("x_sh", [P, J * IN_DIM], F32, kind="ExternalInput")
    gidx_d = nc.dram_tensor("gidx", [P, GC], I16, kind="ExternalInput")
    sseg_d = nc.dram_tensor("sseg", [P, plan["SCOLS"] * 128], FP8,
                            kind="ExternalInput")
    min_d = nc.dram_tensor("mask_in", [P, SD], BF16, kind="ExternalInput")
    mout_d = nc.dram_tensor("mask_out", [P, SDo], BF16, kind="ExternalInput")
    W0_d = nc.dram_tensor("W0", [IN_DIM, H], F32, kind="ExternalInput")
    Wl_d = [nc.dram_tensor(f"W{l}", [H, H], F32, kind="ExternalInput")
            for l in (1, 2, 3)]
    Wfc_d = nc.dram_tensor("Wfc", [H, N_CLASSES], F32, kind="ExternalInput")
    b_d = [nc.dram_tensor(f"b{l}", [1, H], F32, kind="ExternalInput")
           for l in range(4)]
    bfc_d = nc.dram_tensor("bfc", [1, N_CLASSES], F32, kind="ExternalInput")
    out_d = nc.dram_tensor("out", [1, N_CLASSES], F32, kind="ExternalOutput")

    zshard = nc.dram_tensor("zshard", [SLOT, H], BF16, kind="Internal")
    table = nc.dram_tensor("ztable", [NT, H], BF16, kind="Internal",
                           addr_space="Shared")
    gmax_l = nc.dram_tensor("gmax_l", [H, 1], F32, kind="Internal")
    gmax_g = nc.dram_tensor("gmax_g", [H, 1], F32, kind="Internal",
                            addr_space="Shared")

    groups = [list(range(N_CORES))]
    KMAX = [(NT - q) // 4 for q in range(4)]

    with tile.TileContext(nc) as tc:
        import contextlib
        with contextlib.ExitStack() as ctx:
            cpool = ctx.enter_context(tc.tile_pool(name="const", bufs=1))
            bigp = ctx.enter_context(tc.tile_pool(name="big", bufs=1))
            slabp = ctx.enter_context(tc.tile_pool(name="slab", bufs=2))
            sp = ctx.enter_context(tc.tile_pool(name="sseg", bufs=4))
            idxp = ctx.enter_context(tc.tile_pool(name="idx", bufs=3))
            xp = ctx.enter_context(tc.tile_pool(name="xs", bufs=3))
            scr = ctx.enter_context(tc.tile_pool(name="scr", bufs=3))
            psp = ctx.enter_context(tc.tile_pool(name="ps", bufs=2,
                                                 space="PSUM"))
            pmp = ctx.enter_context(tc.tile_pool(name="pm", bufs=2,
                                                 space="PSUM"))

            # ---- constants ----
            ident = cpool.tile([P, P], F32)
            make_identity(nc, ident[:])
            ones_row = cpool.tile([1, P], F32)
            nc.gpsimd.memset(ones_row[:], 1.0)

            W0_t = cpool.tile([IN_DIM, H], F32)
            nc.sync.dma_start(out=W0_t[:], in_=W0_d.ap()[:, :])
            Wl_t = []
            for l in range(3):
                w = cpool.tile([H, H], F32, tag=f"W{l + 1}")
                nc.sync.dma_start(out=w[:], in_=Wl_d[l].ap()[:, :])
                Wl_t.append(w)
            Wfc_t = cpool.tile([H, N_CLASSES], F32)
            nc.sync.dma_start(out=Wfc_t[:], in_=Wfc_d.ap()[:, :])
            bfc_t = cpool.tile([1, N_CLASSES], F32)
            nc.sync.dma_start(out=bfc_t[:], in_=bfc_d.ap()[:, :])

            b_tiles = []
            for l in range(4):
                brow = cpool.tile([1, H], F32, tag=f"brow{l}")
                nc.sync.dma_start(out=brow[:], in_=b_d[l].ap()[:, :])
                bps = psp.tile([P, H], F32, tag="zp")
                nc.tensor.matmul(out=bps[:], lhsT=ones_row[:], rhs=brow[:],
                                 start=True, stop=True)
                bt = cpool.tile([P, H], F32, tag=f"btile{l}")
                nc.vector.tensor_copy(out=bt[:], in_=bps[:])
                b_tiles.append(bt)

            # ---- degrees from masks; ninv = sqrt(1/clip(deg,1)) ----
            def make_ninv(mask_dram, SDx, offx, DPx, tagp):
                mt = bigp.tile([P, SDx], BF16, tag=f"mask{tagp}")
                nc.sync.dma_start(out=mt[:], in_=mask_dram.ap()[:, :])
                deg = bigp.tile([P, J], F32, tag=f"deg{tagp}")
                for j in range(J):
                    nc.vector.reduce_sum(
                        out=deg[:, j:j + 1],
                        in_=mt[:, int(offx[j]):int(offx[j] + DPx[j])],
                        axis=mybir.AxisListType.X)
                nc.vector.tensor_scalar_max(out=deg[:], in0=deg[:],
                                            scalar1=1.0)
                rec = bigp.tile([P, J], F32, tag=f"rec{tagp}")
                nc.vector.reciprocal(out=rec[:], in_=deg[:])
                ninv = bigp.tile([P, J], F32, tag=f"ninv{tagp}")
                nc.scalar.activation(out=ninv[:], in_=rec[:],
                                     func=mybir.ActivationFunctionType.Sqrt)
                return ninv

            ninv_in = make_ninv(min_d, SD, off, DP, "i")
            ninv_out = make_ninv(mout_d, SDo, offo, DPo, "o")

            h_sb = bigp.tile([P, J * H], F32)
            z_sb = bigp.tile([P, J * H], BF16)

            # ---- layer 0 local: z0 = (x * ninv_out) @ W0 ----
            for j in range(J):
                xt_in = xp.tile([P, IN_DIM], F32, tag="xin")
                nc.sync.dma_start(out=xt_in[:],
                                  in_=x_d.ap()[:, j * IN_DIM:(j + 1) * IN_DIM])
                xs = xp.tile([P, IN_DIM], F32, tag="xsc")
                nc.vector.tensor_scalar_mul(out=xs[:], in0=xt_in[:],
                                            scalar1=ninv_out[:, j:j + 1])
                tp = psp.tile([P, P], F32, tag="tp")
                nc.tensor.transpose(out=tp[:], in_=xs[:], identity=ident[:])
                xt = scr.tile([P, IN_DIM], F32, tag="xT")
                nc.vector.tensor_copy(out=xt[:], in_=tp[:])
                zp = psp.tile([P, H], F32, tag="zp")
                nc.tensor.matmul(out=zp[:], lhsT=xt[:], rhs=W0_t[:],
                                 start=True, stop=True)
                nc.scalar.activation(out=z_sb[:, j * H:(j + 1) * H], in_=zp[:],
                                     func=mybir.ActivationFunctionType.Copy)

            zshard_ap = zshard.ap().rearrange("(p j) f -> p (j f)", p=P)
            wins = [table.ap()[q:q + 4 * KMAX[q], :].rearrange(
                "(a b) f -> a (b f)", b=4) for q in range(4)]

            qq = 0
            for layer in range(4):
                # publish z -> all-gather the replicated z-table
                nc.sync.dma_start(out=zshard_ap, in_=z_sb[:])
                nc.gpsimd.collective_compute(
                    "AllGather", mybir.AluOpType.bypass,
                    replica_groups=groups,
                    ins=[zshard.ap()[:, :]],
                    outs=[table.ap()[0:N_CORES * SLOT, :]],
                )

                for (j0, j1, rc0, runs) in range_meta:
                    rcw = (runs[-1][2] // 128) - rc0
                    slab = slabp.tile([P, rcw * 128], BF16, tag="slab")
                    git = idxp.tile([P, rcw * 8], I16, tag="gi")
                    nc.sync.dma_start(
                        out=git[:],
                        in_=gidx_d.ap()[:, rc0 * 8:(rc0 + rcw) * 8])
                    for (q, e0, e1) in runs:
                        total = e1 - e0
                        pos = e0
                        while total > 0:
                            n = min(total, MAX_NI)
                            lc = pos // 128 - rc0
                            sl = slab[:, lc * 128:(lc + n // 128) * 128
                                      ].rearrange("p (s f) -> p s f", f=128)
                            nc.gpsimd.dma_gather(
                                sl, wins[q],
                                git[:, lc * 8:lc * 8 + n // 16],
                                n, n, 128, elem_step=128,
                                queue_num=qq % NQ)
                            qq += 1
                            pos += n
                            total -= n

                    for j in range(j0, j1):
                        cw = int(W[j])
                        st = sp.tile([P, cw * 128], FP8, tag="S")
                        s0 = int(scol0[j]) * 128
                        seng = nc.scalar if j % 2 == 0 else nc.sync
                        seng.dma_start(
                            out=st[:],
                            in_=sseg_d.ap()[:, s0:s0 + cw * 128])
                        pm = pmp.tile([P, H], F32, tag="m")
                        k = 0
                        for (q, cstart, cend) in chunk_cols[j]:
                            for c in range(cstart, cend + 1):
                                lcol = c - rc0
                                nc.tensor.matmul(
                                    out=pm[:],
                                    lhsT=st[:, k * 128:(k + 1) * 128],
                                    rhs=slab[:, lcol * 128:
                                             lcol * 128 + H],
                                    start=(k == 0), stop=(k == cw - 1))
                                k += 1
                        hpre = scr.tile([P, H], F32, tag="hpre")
                        nc.vector.scalar_tensor_tensor(
                            out=hpre[:], in0=pm[:],
                            scalar=ninv_in[:, j:j + 1],
                            in1=b_tiles[layer][:],
                            op0=mybir.AluOpType.mult,
                            op1=mybir.AluOpType.add)
                        nc.scalar.activation(
                            out=h_sb[:, j * H:(j + 1) * H], in_=hpre[:],
                            func=mybir.ActivationFunctionType.Relu)

                        if layer < 3:
                            hs = scr.tile([P, H], F32, tag="hs")
                            nc.vector.tensor_scalar_mul(
                                out=hs[:], in0=h_sb[:, j * H:(j + 1) * H],
                                scalar1=ninv_out[:, j:j + 1])
                            tp = psp.tile([P, P], F32, tag="tp")
                            nc.tensor.transpose(out=tp[:H, :], in_=hs[:],
                                                identity=ident[:])
                            hts = scr.tile([H, P], F32, tag="hts")
                            nc.vector.tensor_copy(out=hts[:], in_=tp[:H, :])
                            zp = psp.tile([P, H], F32, tag="zp")
                            nc.tensor.matmul(out=zp[:], lhsT=hts[:],
                                             rhs=Wl_t[layer][:],
                                             start=True, stop=True)
                            nc.scalar.activation(
                                out=z_sb[:, j * H:(j + 1) * H], in_=zp[:],
                                func=mybir.ActivationFunctionType.Copy)

            # ---- graph max-pool over real nodes (h already relu'd) ----
            pm2 = bigp.tile([H, J], F32)
            for j in range(J):
                tp = psp.tile([P, P], F32, tag="tp")
                nc.tensor.transpose(out=tp[:H, :],
                                    in_=h_sb[:, j * H:(j + 1) * H],
                                    identity=ident[:])
                hts = scr.tile([H, P], F32, tag="hts")
                nc.vector.tensor_copy(out=hts[:], in_=tp[:H, :])
                ncols = P if (j + 1) * P <= REAL else max(0, REAL - j * P)
                if ncols > 0:
                    nc.vector.reduce_max(out=pm2[:H, j:j + 1],
                                         in_=hts[:, :ncols],
                                         axis=mybir.AxisListType.X)
                else:
                    nc.vector.memset(pm2[:H, j:j + 1], -1e30)
            gmax = scr.tile([H, 1], F32, tag="gmax")
            nc.vector.reduce_max(out=gmax[:H, :], in_=pm2[:H, :],
                                 axis=mybir.AxisListType.X)
            nc.sync.dma_start(out=gmax_l.ap()[:, :], in_=gmax[:])
            nc.gpsimd.collective_compute(
                "AllReduce", mybir.AluOpType.max,
                replica_groups=groups,
                ins=[gmax_l.ap()[:, :]],
                outs=[gmax_g.ap()[:, :]],
            )
            g_sb = scr.tile([H, 1], F32, tag="gsb")
            nc.sync.dma_start(out=g_sb[:], in_=gmax_g.ap()[:, :])

            # ---- logits + softmax (replicated on every core) ----
            lgp = psp.tile([1, N_CLASSES], F32, tag="lg")
            nc.tensor.matmul(out=lgp[:], lhsT=g_sb[:H, :], rhs=Wfc_t[:],
                             start=True, stop=True)
            lg = scr.tile([1, N_CLASSES], F32, tag="lg1")
            nc.vector.tensor_copy(out=lg[:], in_=lgp[:])
            lgb = scr.tile([1, N_CLASSES], F32, tag="lg2")
            nc.vector.tensor_add(out=lgb[:], in0=lg[:], in1=bfc_t[:])
            mx = scr.tile([1, 1], F32, tag="mx")
            nc.vector.reduce_max(out=mx[:], in_=lgb[:],
                                 axis=mybir.AxisListType.X)
            sh = scr.tile([1, N_CLASSES], F32, tag="sh")
            nc.vector.tensor_scalar(out=sh[:], in0=lgb[:], scalar1=mx[:],
                                    scalar2=None,
                                    op0=mybir.AluOpType.subtract)
            ex = scr.tile([1, N_CLASSES], F32, tag="ex")
            nc.scalar.activation(out=ex[:], in_=sh[:],
                                 func=mybir.ActivationFunctionType.Exp)
            sm = scr.tile([1, 1], F32, tag="sm")
            nc.vector.reduce_sum(out=sm[:], in_=ex[:],
                                 axis=mybir.AxisListType.X)
            rs = scr.tile([1, 1], F32, tag="rs")
            nc.vector.reciprocal(out=rs[:], in_=sm[:])
            so = scr.tile([1, N_CLASSES], F32, tag="so")
            nc.vector.tensor_scalar_mul(out=so[:], in0=ex[:], scalar1=rs[:])
            nc.sync.dma_start(out=out_d.ap()[:, :], in_=so[:])

    nc.compile()
    return nc


# --------------------------------------------------------------------------
# Host wrapper
# --------------------------------------------------------------------------
def _make_in_maps(plan, x, W0, b0, W1, b1, W2, b2, W3, b3, Wfc, bfc):
    x = np.asarray(x, dtype=np.float32)
    in_maps = []
    pp, jj = np.meshgrid(np.arange(P), np.arange(J), indexing="ij")
    l_grid = jj * 128 + pp
    valid = l_grid < REAL
    for c in range(N_CORES):
        xs = np.zeros((P, J, IN_DIM), dtype=np.float32)
        n_grid = np.where(valid, l_grid * N_CORES + c, 0)
        xs[valid] = x[n_grid[valid]]
        im = {
            "x_sh": xs.reshape(P, J * IN_DIM),
            "gidx": plan["gidx"][c],
            "sseg": plan["sseg"][c],
            "mask_in": plan["mask_in"][c],
            "mask_out": plan["mask_out"][c],
            "W0": np.asarray(W0, dtype=np.float32),
            "W1": np.asarray(W1, dtype=np.float32),
            "W2": np.asarray(W2, dtype=np.float32),
            "W3": np.asarray(W3, dtype=np.float32),
            "Wfc": np.asarray(Wfc, dtype=np.float32),
            "b0": np.asarray(b0, dtype=np.float32).reshape(1, HID),
            "b1": np.asarray(b1, dtype=np.float32).reshape(1, HID),
            "b2": np.asarray(b2, dtype=np.float32).reshape(1, HID),
            "b3": np.asarray(b3, dtype=np.float32).reshape(1, HID),
            "bfc": np.asarray(bfc, dtype=np.float32).reshape(1, N_CLASSES),
        }
        in_maps.append(im)
    return in_maps


def kernel(x, src, dst, W0, b0, W1, b1, W2, b2, W3, b3, Wfc, bfc):
    global LAST_RESULTS, LAST_NC, LAST_IN_MAPS
    x = np.asarray(x, dtype=np.float32)
    assert x.shape == (N_NODES, IN_DIM)
    plan = make_plan(src, dst)
    nc = build_program(plan)
    in_maps = _make_in_maps(plan, x, W0, b0, W1, b1, W2, b2, W3, b3, Wfc, bfc)
    LAST_NC, LAST_IN_MAPS = nc, in_maps
    res = bass_utils.run_bass_kernel_spmd(
        nc, in_maps, core_ids=list(range(N_CORES)),
        trace=bool(os.environ.get("GNN_TRACE")),
    )
    LAST_RESULTS = res
    return np.asarray(res.results[0]["out"], dtype=np.float32)
